# revision 45
# baseline (speedup 1.0000x reference)
"""Trainium2 Bass kernel for nn_Attention_12515534700827.

Multi-head causal attention with RoPE: B=2, S=2048, D=1024, H=16, HD=64.
Sharding: 8 cores = 2 (batch) x 4 (head groups of 4 heads). Each core
computes its 4 heads' attention + its slice of the wo projection; the host
sums the 4 partial outputs per batch (the "all-reduce after wo").

All layout transforms (transposes, head permutations for rotate-half RoPE)
are done host-side in numpy; the device kernel sees matmul-ready layouts.
All matmul operands are bf16 (PE runs bf16 at 1 cycle/row for any moving
size, vs fp32r's 4 cycles/row below N=256); accumulation stays fp32 in
PSUM, softmax exp reads fp32 scores.

Per-core dataflow (pair = 2 heads; 2 pairs per core):
  - x streamed in 512-column chunks so projections start ~3us in and the
    PE stays warm (HAM K=8/8).
  - Q^T,K^T computed directly in [head_dim, seq] layout; RoPE =
    A*C + swap(A)*S with swap done by a PE permutation matmul.
  - scores computed transposed [k, q] (2 heads concurrently via PE row
    tiles), causal k-blocks skipped, diagonal mask added with a bf16
    I.T @ maskT matmul, exp on ScalarE with fused 1/sqrt(hd) scale into
    bf16 probs.
  - attention is software-pipelined (depth 2; depth 1 for the final pair
    so its normalizations emit sooner): scores+exp of k-block kb are
    emitted before the PV of kb-depth, so the in-order PE queue never
    stalls on the ScalarE exp (keeps the PE dense -> HAM at full clock),
    and the previous chunk's wo blocks are spread between early k-blocks
    as PE filler over the normalization-chain wait.
  - PV: probsT [k,q] moving, V' = [V_A|ones|V_B|ones] stationary (M=65
    per head, fused softmax denominator row), accumulated per 512-col sub
    so normalization starts as soon as a sub's causal k-blocks finish.
  - normalization: reciprocal of the denominator row, DMA partition-
    broadcast, multiply into bf16 attnT tiles.
  - wo: emitted one chunk behind attention (after the next chunk's first
    pair) so its dependency on the normalization DMA chain never blocks
    the PE queue; output DMA'd to HBM as bf16, host sums partials in fp32.

Tried and rejected: fp8 DoubleRow projections (V-path quantization alone
gives 4.7e-2 max-rel err -- early queries have a peaked softmax so fp8
noise doesn't average out); diag-mask on VectorE (cross-engine hop stalls
the score pipeline, +43us); PV half-K row-tiled pairs (two in-flight MMs
accumulating the same PSUM bank corrupt results on HW).
"""

import sys

if "/opt/trn_rl_repo" not in sys.path:
    sys.path.insert(0, "/opt/trn_rl_repo")

import numpy as np

import concourse.mybir as mybir
import concourse.tile as tile
from concourse import bacc
from concourse.bass_utils import run_bass_kernel_spmd

F32 = mybir.dt.float32
F8 = mybir.dt.float8e4
BF16 = mybir.dt.bfloat16
AF = mybir.ActivationFunctionType

B, S, D, H, HD = 2, 2048, 1024, 16, 64
NCORES = 8
GROUPS = 4            # head groups (cores per batch)
HPG = H // GROUPS     # heads per core = 4
NPAIR = HPG // 2      # head pairs per core = 2
NEG_INF = -1e9
SM_SCALE = 1.0 / float(np.sqrt(HD))  # 0.125

NIT = D // 128        # 8 contraction tiles
NSB = S // 128        # 16 seq blocks
NC512 = S // 512      # 4
QCH = 1024            # attention q-chunk
NCHUNK = S // QCH     # 2

_PROG_CACHE = {}


def _build_program(mask_kind: str, debug: bool = False):
    """mask_kind: 'causal' (skip + diag mask) or 'zeros' (full, no mask)."""
    causal = mask_kind == "causal"
    nc = bacc.Bacc("TRN2", target_bir_lowering=False, debug=False,
                   num_devices=NCORES)
    dbg = {}

    xT_d = nc.dram_tensor("xT", [D, S], BF16, kind="ExternalInput").ap()
    wqT_d = nc.dram_tensor("wqT", [D, HPG * HD], BF16, kind="ExternalInput").ap()
    wkT_d = nc.dram_tensor("wkT", [D, HPG * HD], BF16, kind="ExternalInput").ap()
    wvT_d = nc.dram_tensor("wvT", [D, HPG * HD], BF16, kind="ExternalInput").ap()
    woT_d = nc.dram_tensor("woT", [HPG * HD, D], BF16, kind="ExternalInput").ap()
    c_d = nc.dram_tensor("c128", [128, S], BF16, kind="ExternalInput").ap()
    s_d = nc.dram_tensor("s128", [128, S], BF16, kind="ExternalInput").ap()
    pmat_d = nc.dram_tensor("pmat", [128, 128], BF16, kind="ExternalInput").ap()
    ident_d = nc.dram_tensor("ident", [128, 128], BF16, kind="ExternalInput").ap()
    mdiag_d = nc.dram_tensor("mdiagT", [128, 128], BF16, kind="ExternalInput").ap()
    out_d = nc.dram_tensor("out", [S, D], BF16, kind="ExternalOutput").ap()
    if debug:
        for nm, shp, dt in [("dqt", [128, S], BF16), ("dkt", [128, S], BF16),
                            ("dvp", [128, 130], BF16), ("dpt", [128, QCH], BF16),
                            ("dov", [65, QCH], F32), ("dat0", [128, S], BF16),
                            ("dat1", [128, S], BF16)]:
            dbg[nm] = nc.dram_tensor(nm, shp, dt, kind="ExternalOutput").ap()

    with tile.TileContext(nc) as tc:
        from contextlib import ExitStack

        with ExitStack() as root:
            pers = root.enter_context(tc.tile_pool(name="pers", bufs=1))

            # ---- persistent SBUF tiles ----
            qt = [pers.tile([128, S], BF16, tag=f"qt{p}", name=f"qt{p}") for p in range(NPAIR)]
            kt = [pers.tile([128, S], BF16, tag=f"kt{p}", name=f"kt{p}") for p in range(NPAIR)]
            # V' per (pair, s-block): [128,130] = V_A|ones|V_B|ones
            vp = [[pers.tile([128, 130], BF16, tag=f"vp{p}_{sb}", name=f"vp{p}_{sb}")
                   for sb in range(NSB)] for p in range(NPAIR)]
            at = [pers.tile([128, S], BF16, tag=f"at{p}", name=f"at{p}") for p in range(NPAIR)]
            wo_t = [pers.tile([128, D], BF16, tag=f"wo{p}", name=f"wo{p}")
                    for p in range(NPAIR)]
            ident_t = pers.tile([128, 128], BF16, tag="ident", name="ident")
            mdiag_t = pers.tile([128, 128], BF16, tag="mdiag", name="mdiag")

            # ---- phase B pools (freed before attention) ----
            with ExitStack() as phb:
                ld = phb.enter_context(tc.tile_pool(name="ld", bufs=1))
                xts = [ld.tile([128, S], BF16, tag=f"xt{it}", name=f"xt{it}") for it in range(NIT)]
                wq_t = [ld.tile([128, HPG * HD], BF16, tag=f"wq{it}", name=f"wq{it}") for it in range(NIT)]
                wk_t = [ld.tile([128, HPG * HD], BF16, tag=f"wk{it}", name=f"wk{it}") for it in range(NIT)]
                wv_t = [ld.tile([128, HPG * HD], BF16, tag=f"wv{it}", name=f"wv{it}") for it in range(NIT)]
                c_t = ld.tile([128, S], BF16, tag="c128", name="c128")
                s_t = ld.tile([128, S], BF16, tag="s128", name="s128")
                pm_t = ld.tile([128, 128], BF16, tag="pmat", name="pmat")

                # DMA issue order tuned so the first QK projection chunk can
                # start ~3us in: q/k weights + rope tables, x chunk 0, then
                # the rest.
                for it in range(NIT):
                    sl = slice(it * 128, (it + 1) * 128)
                    nc.sync.dma_start(out=wq_t[it][:], in_=wqT_d[sl, :])
                    nc.sync.dma_start(out=xts[it][:, 0:512], in_=xT_d[sl, 0:512])
                for it in range(NIT):
                    sl = slice(it * 128, (it + 1) * 128)
                    nc.sync.dma_start(out=wk_t[it][:], in_=wkT_d[sl, :])
                nc.sync.dma_start(out=c_t[:], in_=c_d[:])
                nc.sync.dma_start(out=s_t[:], in_=s_d[:])
                nc.sync.dma_start(out=pm_t[:], in_=pmat_d[:])
                for it in range(NIT):
                    sl = slice(it * 128, (it + 1) * 128)
                    nc.sync.dma_start(out=wv_t[it][:], in_=wvT_d[sl, :])
                nc.sync.dma_start(out=ident_t[:], in_=ident_d[:])
                if causal:
                    nc.sync.dma_start(out=mdiag_t[:], in_=mdiag_d[:])
                for c in range(1, NC512):
                    qs = slice(c * 512, (c + 1) * 512)
                    for it in range(NIT):
                        sl = slice(it * 128, (it + 1) * 128)
                        nc.sync.dma_start(out=xts[it][:, qs], in_=xT_d[sl, qs])
                for p in range(NPAIR):
                    nc.sync.dma_start(
                        out=wo_t[p][:], in_=woT_d[p * 128:(p + 1) * 128, :])

                # ones columns of V' via vector memset
                for p in range(NPAIR):
                    for sb in range(NSB):
                        nc.vector.memset(vp[p][sb][:, 64:65], 1.0)
                        nc.vector.memset(vp[p][sb][:, 129:130], 1.0)

                psA = phb.enter_context(
                    tc.tile_pool(name="psA", bufs=2, space="PSUM"))
                psSW = phb.enter_context(
                    tc.tile_pool(name="psSW", bufs=2, space="PSUM"))
                psV = phb.enter_context(
                    tc.tile_pool(name="psV", bufs=2, space="PSUM"))
                sbA = phb.enter_context(tc.tile_pool(name="sbA", bufs=3))

                # x-chunk-major so each arriving chunk unlocks its work
                for c in range(NC512):
                    qs = slice(c * 512, (c + 1) * 512)
                    # Q/K projections (transposed layout) + rope
                    for wt, dst in ((wq_t, qt), (wk_t, kt)):
                        for p in range(NPAIR):
                            pc = slice(p * 128, (p + 1) * 128)
                            # col-split: the two M=64 halves of each
                            # projection matmul run concurrently on the PE
                            # (col groups 0 and 64). They write different
                            # banks AND different partition ranges of one
                            # [128,1024] tile so each half has its own
                            # accumulation-group zero region.
                            ps = psA.tile([128, 1024], F32, tag="psA", name="psA")
                            for it in range(NIT):
                                for hf in range(2):
                                    nc.tensor.matmul(
                                        ps[hf * 64:(hf + 1) * 64,
                                           hf * 512:(hf + 1) * 512],
                                        wt[it][:, p * 128 + hf * 64:
                                               p * 128 + (hf + 1) * 64],
                                        xts[it][:, qs],
                                        start=(it == 0), stop=(it == NIT - 1))
                            # rope: rot = A*C + swap(A)*S
                            a_sb = sbA.tile([128, 512], BF16, tag="sbA", name="sbA")
                            nc.scalar.copy(a_sb[0:64, :], ps[0:64, 0:512])
                            nc.scalar.copy(a_sb[64:128, :], ps[64:128, 512:1024])
                            sw = psSW.tile([128, 512], F32, tag="psSW", name="psSW")
                            nc.tensor.matmul(sw[:], pm_t[:], a_sb[:],
                                             start=True, stop=True)
                            t1 = sbA.tile([128, 512], BF16, tag="t1", name="t1")
                            nc.vector.tensor_mul(t1[:], a_sb[:], c_t[:, qs])
                            t2 = sbA.tile([128, 512], BF16, tag="t2", name="t2")
                            nc.vector.tensor_mul(t2[:], sw[:], s_t[:, qs])
                            nc.vector.tensor_add(dst[p][:, qs], t1[:], t2[:])
                    # V projection (natural layout) into V' tiles
                    for j in range(4):
                        sb = c * 4 + j
                        ssl = slice(sb * 128, (sb + 1) * 128)
                        ps = psV.tile([128, HPG * HD], F32, tag="psV", name="psV")
                        for it in range(NIT):
                            nc.tensor.matmul(ps[:], xts[it][:, ssl], wv_t[it][:],
                                             start=(it == 0), stop=(it == NIT - 1))
                        for p in range(NPAIR):
                            src = ps[:, p * 128:(p + 1) * 128]
                            nc.vector.tensor_copy(vp[p][sb][:, 0:64], src[:, 0:64])
                            nc.vector.tensor_copy(vp[p][sb][:, 65:129], src[:, 64:128])

            if debug:
                nc.sync.dma_start(out=dbg["dqt"][:], in_=qt[0][:])
                nc.sync.dma_start(out=dbg["dkt"][:], in_=kt[0][:])
                nc.sync.dma_start(out=dbg["dvp"][:], in_=vp[0][0][:])

            # ================= Attention + wo, chunk-interleaved ==========
            with ExitStack() as phc:
                psS = phc.enter_context(
                    tc.tile_pool(name="psS", bufs=2, space="PSUM"))
                psO = phc.enter_context(
                    tc.tile_pool(name="psO", bufs=1, space="PSUM"))
                prb = phc.enter_context(tc.tile_pool(name="prb", bufs=6))
                nrm = phc.enter_context(tc.tile_pool(name="nrm", bufs=3))
                osb = phc.enter_context(tc.tile_pool(name="osb", bufs=4))
                drp = phc.enter_context(
                    tc.tile_pool(name="drp", bufs=4, space="DRAM"))

                def norm_sub(p, q0, ov, h, sub):
                    """attnT[:, sub] = ov_data * recip(denom row 64)."""
                    s0 = q0 + sub * 512
                    dr = drp.tile([1, 512], F32, tag="dr", name="dr")
                    rr = nrm.tile([65, 512], F32, tag="rr", name="rr")
                    # single-partition custom-DVE ops at base 64 misbehave
                    # on HW: copy the denominator row out, broadcast it,
                    # then reciprocal on 64 partitions
                    nc.vector.tensor_copy(rr[64:65, :], ov[h][sub][64:65, :])
                    nc.sync.dma_start(out=dr[:], in_=rr[64:65, :])
                    rbc = nrm.tile([64, 512], F32, tag="rbc", name="rbc")
                    nc.sync.dma_start(
                        out=rbc[:], in_=dr[:].to_broadcast((64, 512)))
                    rrec = nrm.tile([64, 512], F32, tag="rrec", name="rrec")
                    nc.vector.reciprocal_approx_fast(rrec[:], rbc[:])
                    if h == 0:
                        nc.vector.tensor_mul(
                            at[p][0:64, s0:s0 + 512], ov[h][sub][0:64, :],
                            rrec[:])
                    else:
                        atb = nrm.tile([64, 512], BF16, tag="atb", name="atb")
                        nc.vector.tensor_mul(atb[:], ov[h][sub][0:64, :],
                                             rrec[:])
                        nc.sync.dma_start(
                            out=at[p][64:128, s0:s0 + 512], in_=atb[:])

                def wo_block(sb):
                    """out[sb*128:(sb+1)*128, :] = sum_p at[p].T @ wo_p.
                    PSUM comes from the scores tag (one 512-wide bank per
                    output half)."""
                    ssl = slice(sb * 128, (sb + 1) * 128)
                    ps = psS.tile([128, QCH], F32, tag="sc", name="scW")
                    for oc in range(2):
                        osl = slice(oc * 512, (oc + 1) * 512)
                        for p in range(NPAIR):
                            nc.tensor.matmul(
                                ps[:, osl], at[p][:, ssl], wo_t[p][:, osl],
                                start=(p == 0), stop=(p == NPAIR - 1))
                    ob = osb.tile([128, 1024], BF16, tag="osb", name="osb")
                    nc.vector.tensor_copy(ob[:], ps[:])
                    nc.sync.dma_start(out=out_d[ssl, :], in_=ob[:])

                def attention(c, p, tail_wo, wo_list):
                    """Attention for (chunk, pair), depth-1 software
                    pipelined: scores+exp of k-block kb are emitted BEFORE
                    the PV of k-block kb-1, so the in-order PE queue never
                    stalls on the ScalarE exp. PV runs as row-tiled half-K
                    pairs: head0's half targets ov0 while head1's other half
                    targets ov1 concurrently (disjoint row groups + banks).
                    Normalization of each 512-col sub is emitted right after
                    the k-block that completes it."""
                    q0 = c * QCH
                    kb_hi = (c * 8 + 8) if causal else NSB
                    nsub = QCH // 512
                    ov = [[psO.tile([65, 512], F32, tag=f"ov{h}_{s}",
                                    name=f"ov{h}_{s}") for s in range(nsub)]
                          for h in range(2)]
                    last_for = []
                    for sub in range(nsub):
                        if causal:
                            last_for.append(
                                min(kb_hi, (q0 + (sub + 1) * 512) // 128) - 1)
                        else:
                            last_for.append(kb_hi - 1)

                    def emit_scores(kb):
                        k0 = kb * 128
                        trim = max(q0, k0) if causal else q0
                        on_diag = causal and kb >= c * (QCH // 128)
                        pts = [None, None]
                        for h in range(2):
                            hsl = slice(h * 64, (h + 1) * 64)
                            sc = psS.tile([128, QCH], F32, tag="sc", name="sc")
                            # each 512-col sub-MM opens its own PSUM-bank
                            # accumulation group; the diag-mask matmul closes
                            # the group of the bank it lands in
                            diag_sub = (k0 - q0) // 512 if on_diag else -1
                            for sub in range(nsub):
                                a = max(q0 + sub * 512, trim)
                                b_ = q0 + sub * 512 + 512
                                if a >= b_:
                                    continue
                                nc.tensor.matmul(
                                    sc[:, a - q0:b_ - q0],
                                    kt[p][hsl, k0:k0 + 128],
                                    qt[p][hsl, a:b_],
                                    start=True, stop=(sub != diag_sub))
                            if on_diag:
                                # additive causal mask on diag subblock
                                nc.tensor.matmul(
                                    sc[:, k0 - q0:k0 - q0 + 128],
                                    ident_t[:], mdiag_t[:],
                                    start=False, stop=True)
                            # exp (with fused 1/sqrt(hd) scale) -> bf16
                            pt = prb.tile([128, QCH], BF16, tag="prb", name="prb")
                            nc.scalar.activation(
                                pt[:, trim - q0:], sc[:, trim - q0:],
                                AF.Exp, scale=SM_SCALE)
                            pts[h] = pt
                            if debug and p == 0 and c == 0 and kb == 0 \
                                    and h == 0:
                                nc.sync.dma_start(out=dbg["dpt"][:], in_=pt[:])
                        return kb, trim, pts

                    def emit_pv(st):
                        kb, trim, pts = st
                        for sub in range(nsub):
                            a = max(q0 + sub * 512, trim)
                            b_ = q0 + sub * 512 + 512
                            if a >= b_:
                                continue
                            s0 = q0 + sub * 512
                            first = kb == 0
                            last = kb == last_for[sub]
                            for h in range(2):
                                # PV + denominator (M=65: V_h | ones)
                                nc.tensor.matmul(
                                    ov[h][sub][:, a - s0:b_ - s0],
                                    vp[p][kb][:, h * 65:h * 65 + 65],
                                    pts[h][:, a - q0:b_ - q0],
                                    start=first, stop=last)
                        for sub in range(nsub):
                            if kb == last_for[sub]:
                                if debug and p == 0 and c == 0 and sub == 0:
                                    ovb = nrm.tile([65, 512], F32, tag="ovb",
                                                   name="ovb")
                                    nc.vector.tensor_copy(ovb[:], ov[0][sub][:])
                                    nc.sync.dma_start(
                                        out=dbg["dov"][:, 0:512], in_=ovb[:])
                                norm_sub(p, q0, ov, 0, sub)
                                norm_sub(p, q0, ov, 1, sub)

                    # depth-2 pipeline: PV trails scores by two k-blocks so
                    # the PE queue has cover for the previous chunk's
                    # normalization chain; wo blocks of the previous chunk
                    # are spread between early k-blocks as extra PE filler.
                    depth = 1 if tail_wo else 2
                    pend = []
                    for kb in range(kb_hi):
                        st = emit_scores(kb)
                        if wo_list and kb % 2 == 0:
                            wo_block(wo_list.pop(0))
                        if len(pend) == depth:
                            emit_pv(pend.pop(0))
                        pend.append(st)
                    while pend:
                        emit_pv(pend.pop(0))
                    while wo_list:
                        wo_block(wo_list.pop(0))
                    if tail_wo:
                        # last chunk: emit wo per half so the first half's
                        # blocks don't wait on the second half's norm chain
                        for sub in range(nsub):
                            for j in range(4):
                                wo_block((q0 + sub * 512) // 128 + j)

                for c in range(NCHUNK):
                    last = c == NCHUNK - 1
                    prev = (list(range((c - 1) * (QCH // 128),
                                       c * (QCH // 128)))
                            if c > 0 else [])
                    attention(c, 0, tail_wo=False, wo_list=prev)
                    attention(c, 1, tail_wo=last, wo_list=[])
                    if last:
                        if debug:
                            nc.sync.dma_start(out=dbg["dat0"][:], in_=at[0][:])
                            nc.sync.dma_start(out=dbg["dat1"][:], in_=at[1][:])

    nc.compile()
    return nc


WSCALE = 64.0  # fp8 weight scale: q,k,v carry x64; folded into exp scale / wo


def _pair(a):
    """[D, M] -> [D//256][128, 2, M] DoubleRow it-tile pair layout."""
    d, m = a.shape
    return np.ascontiguousarray(
        a.reshape(d // 256, 2, 128, m).transpose(0, 2, 1, 3))


def _host_prep(x, freqs_cos, freqs_sin, wq, wk, wv, wo):
    """Build the 8 per-core input maps (all numpy, bf16 via float32 rounds)."""
    import ml_dtypes

    bf = ml_dtypes.bfloat16
    f8 = ml_dtypes.float8_e4m3
    x = np.ascontiguousarray(x, dtype=np.float32)
    cosT = np.ascontiguousarray(freqs_cos.T, dtype=np.float32)  # [32, S]
    sinT = np.ascontiguousarray(freqs_sin.T, dtype=np.float32)

    c128 = np.tile(cosT, (4, 1)).astype(bf)                     # [128, S]
    s128 = np.tile(np.concatenate([-sinT, sinT], 0), (2, 1)).astype(bf)
    # swap permutation: psum_sw = pmat.T @ A -> sw[m] = A[sigma(m)],
    # sigma swaps the 32-halves within each 64 block.
    pmat = np.zeros((128, 128), dtype=np.float32)
    for m in range(128):
        blk, off = divmod(m, 32)
        pmat[(blk ^ 1) * 32 + off, m] = 1.0
    pmat = pmat.astype(bf)
    ident = np.eye(128, dtype=np.float32).astype(bf)
    # causal diag mask, transposed: mdiagT[k, q] = 0 if k <= q else -1e9
    kk, qq = np.meshgrid(np.arange(128), np.arange(128), indexing="ij")
    mdiagT = np.where(kk <= qq, 0.0, NEG_INF).astype(np.float32).astype(bf)

    # rotate-half row permutation within each head
    rh = np.concatenate([np.arange(0, HD, 2), np.arange(1, HD, 2)])

    xT = [np.ascontiguousarray(x[b].T).astype(bf) for b in range(B)]

    in_maps = []
    for core in range(NCORES):
        b, g = divmod(core, GROUPS)
        heads = [g * HPG + j for j in range(HPG)]
        qrows, vrows = [], []
        for h in heads:
            base = h * HD
            qrows.extend((base + rh).tolist())
            vrows.extend(range(base, base + HD))
        qrows = np.array(qrows)
        vrows = np.array(vrows)
        wqT = np.ascontiguousarray(wq[qrows, :].T).astype(bf)   # [D, 256]
        wkT = np.ascontiguousarray(wk[qrows, :].T).astype(bf)
        wvT = np.ascontiguousarray(wv[vrows, :].T).astype(bf)
        woT = np.ascontiguousarray(wo[:, vrows].T).astype(bf)
        m = {
            "xT": xT[b], "wqT": wqT, "wkT": wkT, "wvT": wvT,
            "woT": woT, "c128": c128, "s128": s128, "pmat": pmat,
            "ident": ident, "mdiagT": mdiagT,
        }
        in_maps.append(m)
    return in_maps


def _mask_kind(mask):
    m = np.asarray(mask).reshape(S, S)
    if not np.any(m):
        return "zeros"
    qq, kk = np.meshgrid(np.arange(S), np.arange(S), indexing="ij")
    causal = np.where(kk <= qq, 0.0, NEG_INF).astype(np.float32)  # [q, k]
    if np.array_equal(m, causal):
        return "causal"
    return "general"


def _reference_host(x, freqs_cos, freqs_sin, mask, wq, wk, wv, wo):
    """Correctness fallback for arbitrary masks (host numpy, float64)."""
    b, s, d = x.shape
    hd = d // H
    xq = (x @ wq.T).reshape(b, s, H, hd)
    xk = (x @ wk.T).reshape(b, s, H, hd)
    xv = (x @ wv.T).reshape(b, s, H, hd)

    def rope(t):
        tr = t.reshape(b, s, H, hd // 2, 2)
        t0, t1 = tr[..., 0], tr[..., 1]
        cos = freqs_cos[None, :, None, :]
        sin = freqs_sin[None, :, None, :]
        return np.stack([t0 * cos - t1 * sin, t0 * sin + t1 * cos],
                        -1).reshape(b, s, H, hd)

    xq, xk = rope(xq), rope(xk)
    sc = np.einsum("bqhd,bkhd->bhqk", xq, xk) / np.sqrt(hd) + mask
    sc = sc - sc.max(-1, keepdims=True)
    e = np.exp(sc)
    pr = e / e.sum(-1, keepdims=True)
    o = np.einsum("bhqk,bkhd->bqhd", pr, xv).reshape(b, s, d)
    return (o @ wo.T).astype(np.float32)


def kernel(x, freqs_cos, freqs_sin, mask, wq, wk, wv, wo):
    kind = _mask_kind(mask)
    if kind == "general":
        return _reference_host(np.asarray(x, np.float64),
                               np.asarray(freqs_cos, np.float64),
                               np.asarray(freqs_sin, np.float64),
                               np.asarray(mask, np.float64),
                               np.asarray(wq, np.float64),
                               np.asarray(wk, np.float64),
                               np.asarray(wv, np.float64),
                               np.asarray(wo, np.float64))

    if kind not in _PROG_CACHE:
        _PROG_CACHE[kind] = _build_program(kind)
    nc = _PROG_CACHE[kind]

    in_maps = _host_prep(np.asarray(x, np.float32),
                         np.asarray(freqs_cos, np.float32),
                         np.asarray(freqs_sin, np.float32),
                         np.asarray(wq, np.float32),
                         np.asarray(wk, np.float32),
                         np.asarray(wv, np.float32),
                         np.asarray(wo, np.float32))
    res = run_bass_kernel_spmd(nc, in_maps, list(range(NCORES)))
    out = np.zeros((B, S, D), dtype=np.float32)
    for core in range(NCORES):
        out[core // GROUPS] += np.asarray(res.results[core]["out"],
                                          dtype=np.float32)
    return out


# revision 47
# speedup vs baseline: 1.0615x; 1.0615x over previous
"""Trainium2 Bass kernel for nn_Attention_12515534700827.

Multi-head causal attention with RoPE: B=2, S=2048, D=1024, H=16, HD=64.
Sharding: 8 cores = 2 (batch) x 4 (head groups of 4 heads). Each core
computes its 4 heads' attention + its slice of the wo projection; the host
sums the 4 partial outputs per batch (the "all-reduce after wo").

All layout transforms (transposes, head permutations for rotate-half RoPE)
are done host-side in numpy; the device kernel sees matmul-ready layouts.
All matmul operands are bf16 (PE runs bf16 at 1 cycle/row for any moving
size, vs fp32r's 4 cycles/row below N=256); accumulation stays fp32 in
PSUM, softmax exp reads fp32 scores.

Per-core dataflow (pair = 2 heads; 2 pairs per core):
  - x streamed in 512-column chunks so projections start ~3us in and the
    PE stays warm (HAM K=8/8).
  - Q^T,K^T computed directly in [head_dim, seq] layout; RoPE =
    A*C + swap(A)*S with swap done by a PE permutation matmul.
  - scores computed transposed [k, q] (2 heads concurrently via PE row
    tiles), causal k-blocks skipped, diagonal mask added with a bf16
    I.T @ maskT matmul, exp on ScalarE with fused 1/sqrt(hd) scale into
    bf16 probs.
  - attention is software-pipelined (depth 2; depth 1 for the final pair
    so its normalizations emit sooner): scores+exp of k-block kb are
    emitted before the PV of kb-depth, so the in-order PE queue never
    stalls on the ScalarE exp (keeps the PE dense -> HAM at full clock),
    and the previous chunk's wo blocks are spread between early k-blocks
    as PE filler over the normalization-chain wait.
  - PV: probsT [k,q] moving, V' = [V_A|ones|V_B|ones] stationary (M=65
    per head, fused softmax denominator row), accumulated per 512-col sub
    so normalization starts as soon as a sub's causal k-blocks finish.
  - normalization: reciprocal of the denominator row, DMA partition-
    broadcast, multiply into bf16 attnT tiles.
  - wo: emitted one chunk behind attention (after the next chunk's first
    pair) so its dependency on the normalization DMA chain never blocks
    the PE queue; output DMA'd to HBM as bf16, host sums partials in fp32.

Tried and rejected: fp8 DoubleRow projections (V-path quantization alone
gives 4.7e-2 max-rel err -- early queries have a peaked softmax so fp8
noise doesn't average out); diag-mask on VectorE (cross-engine hop stalls
the score pipeline, +43us); PV half-K row-tiled pairs (two in-flight MMs
accumulating the same PSUM bank corrupt results on HW).
"""

import sys

if "/opt/trn_rl_repo" not in sys.path:
    sys.path.insert(0, "/opt/trn_rl_repo")

import numpy as np

import concourse.mybir as mybir
import concourse.tile as tile
from concourse import bacc
from concourse.bass_utils import run_bass_kernel_spmd

F32 = mybir.dt.float32
F8 = mybir.dt.float8e4
BF16 = mybir.dt.bfloat16
AF = mybir.ActivationFunctionType

B, S, D, H, HD = 2, 2048, 1024, 16, 64
NCORES = 8
GROUPS = 4            # head groups (cores per batch)
HPG = H // GROUPS     # heads per core = 4
NPAIR = HPG // 2      # head pairs per core = 2
NEG_INF = -1e9
SM_SCALE = 1.0 / float(np.sqrt(HD))  # 0.125

NIT = D // 128        # 8 contraction tiles
NSB = S // 128        # 16 seq blocks
NC512 = S // 512      # 4
QCH = 1024            # attention q-chunk
NCHUNK = S // QCH     # 2

_PROG_CACHE = {}


def _build_program(mask_kind: str, debug: bool = False):
    """mask_kind: 'causal' (skip + diag mask) or 'zeros' (full, no mask)."""
    causal = mask_kind == "causal"
    nc = bacc.Bacc("TRN2", target_bir_lowering=False, debug=False,
                   num_devices=NCORES)
    dbg = {}

    xT_d = nc.dram_tensor("xT", [D, S], BF16, kind="ExternalInput").ap()
    wqT_d = nc.dram_tensor("wqT", [D, HPG * HD], BF16, kind="ExternalInput").ap()
    wkT_d = nc.dram_tensor("wkT", [D, HPG * HD], BF16, kind="ExternalInput").ap()
    wvT_d = nc.dram_tensor("wvT", [D, HPG * HD], BF16, kind="ExternalInput").ap()
    woT_d = nc.dram_tensor("woT", [HPG * HD, D], BF16, kind="ExternalInput").ap()
    c_d = nc.dram_tensor("c128", [128, S], BF16, kind="ExternalInput").ap()
    s_d = nc.dram_tensor("s128", [128, S], BF16, kind="ExternalInput").ap()
    pmat_d = nc.dram_tensor("pmat", [128, 128], BF16, kind="ExternalInput").ap()
    ident_d = nc.dram_tensor("ident", [128, 128], BF16, kind="ExternalInput").ap()
    mdiag_d = nc.dram_tensor("mdiagT", [128, 128], BF16, kind="ExternalInput").ap()
    out_d = nc.dram_tensor("out", [S, D], BF16, kind="ExternalOutput").ap()
    if debug:
        for nm, shp, dt in [("dqt", [128, S], BF16), ("dkt", [128, S], BF16),
                            ("dvp", [128, 130], BF16), ("dpt", [128, QCH], BF16),
                            ("dov", [65, QCH], F32), ("dat0", [128, S], BF16),
                            ("dat1", [128, S], BF16)]:
            dbg[nm] = nc.dram_tensor(nm, shp, dt, kind="ExternalOutput").ap()

    with tile.TileContext(nc) as tc:
        from contextlib import ExitStack

        with ExitStack() as root:
            pers = root.enter_context(tc.tile_pool(name="pers", bufs=1))

            # ---- persistent SBUF tiles ----
            qt = [pers.tile([128, S], BF16, tag=f"qt{p}", name=f"qt{p}") for p in range(NPAIR)]
            kt = [pers.tile([128, S], BF16, tag=f"kt{p}", name=f"kt{p}") for p in range(NPAIR)]
            # V' per (pair, s-block): [128,130] = V_A|ones|V_B|ones
            vp = [[pers.tile([128, 130], BF16, tag=f"vp{p}_{sb}", name=f"vp{p}_{sb}")
                   for sb in range(NSB)] for p in range(NPAIR)]
            at = [pers.tile([128, S], BF16, tag=f"at{p}", name=f"at{p}") for p in range(NPAIR)]
            wo_t = [pers.tile([128, D], BF16, tag=f"wo{p}", name=f"wo{p}")
                    for p in range(NPAIR)]
            ident_t = pers.tile([128, 128], BF16, tag="ident", name="ident")
            mdiag_t = pers.tile([128, 128], BF16, tag="mdiag", name="mdiag")

            # ---- phase B pools (freed before attention) ----
            with ExitStack() as phb:
                ld = phb.enter_context(tc.tile_pool(name="ld", bufs=1))
                xts = [ld.tile([128, S], BF16, tag=f"xt{it}", name=f"xt{it}") for it in range(NIT)]
                wq_t = [ld.tile([128, HPG * HD], BF16, tag=f"wq{it}", name=f"wq{it}") for it in range(NIT)]
                wk_t = [ld.tile([128, HPG * HD], BF16, tag=f"wk{it}", name=f"wk{it}") for it in range(NIT)]
                wv_t = [ld.tile([128, HPG * HD], BF16, tag=f"wv{it}", name=f"wv{it}") for it in range(NIT)]
                c_t = ld.tile([128, S], BF16, tag="c128", name="c128")
                s_t = ld.tile([128, S], BF16, tag="s128", name="s128")
                pm_t = ld.tile([128, 128], BF16, tag="pmat", name="pmat")

                # DMA issue order tuned so the first QK projection chunk can
                # start ~3us in: q/k weights + rope tables, x chunk 0, then
                # the rest.
                for it in range(NIT):
                    sl = slice(it * 128, (it + 1) * 128)
                    nc.sync.dma_start(out=wq_t[it][:], in_=wqT_d[sl, :])
                    nc.sync.dma_start(out=xts[it][:, 0:512], in_=xT_d[sl, 0:512])
                for it in range(NIT):
                    sl = slice(it * 128, (it + 1) * 128)
                    nc.sync.dma_start(out=wk_t[it][:], in_=wkT_d[sl, :])
                nc.sync.dma_start(out=c_t[:], in_=c_d[:])
                nc.sync.dma_start(out=s_t[:], in_=s_d[:])
                nc.sync.dma_start(out=pm_t[:], in_=pmat_d[:])
                for it in range(NIT):
                    sl = slice(it * 128, (it + 1) * 128)
                    nc.sync.dma_start(out=wv_t[it][:], in_=wvT_d[sl, :])
                nc.sync.dma_start(out=ident_t[:], in_=ident_d[:])
                if causal:
                    nc.sync.dma_start(out=mdiag_t[:], in_=mdiag_d[:])
                for c in range(1, NC512):
                    qs = slice(c * 512, (c + 1) * 512)
                    for it in range(NIT):
                        sl = slice(it * 128, (it + 1) * 128)
                        nc.sync.dma_start(out=xts[it][:, qs], in_=xT_d[sl, qs])
                for p in range(NPAIR):
                    nc.sync.dma_start(
                        out=wo_t[p][:], in_=woT_d[p * 128:(p + 1) * 128, :])

                # ones columns of V' via vector memset
                for p in range(NPAIR):
                    for sb in range(NSB):
                        nc.vector.memset(vp[p][sb][:, 64:65], 1.0)
                        nc.vector.memset(vp[p][sb][:, 129:130], 1.0)

                psA = phb.enter_context(
                    tc.tile_pool(name="psA", bufs=3, space="PSUM"))
                psSW = phb.enter_context(
                    tc.tile_pool(name="psSW", bufs=2, space="PSUM"))
                psV = phb.enter_context(
                    tc.tile_pool(name="psV", bufs=3, space="PSUM"))
                sbA = phb.enter_context(tc.tile_pool(name="sbA", bufs=3))

                # x-chunk-major so each arriving chunk unlocks its work
                for c in range(NC512):
                    qs = slice(c * 512, (c + 1) * 512)
                    # Q/K projections (transposed layout) + rope
                    for wt, dst in ((wq_t, qt), (wk_t, kt)):
                        for p in range(NPAIR):
                            pc = slice(p * 128, (p + 1) * 128)
                            ps = psA.tile([128, 512], F32, tag="psA", name="psA")
                            for it in range(NIT):
                                nc.tensor.matmul(
                                    ps[:], wt[it][:, pc], xts[it][:, qs],
                                    start=(it == 0), stop=(it == NIT - 1))
                            # rope: rot = A*C + swap(A)*S
                            a_sb = sbA.tile([128, 512], BF16, tag="sbA", name="sbA")
                            nc.scalar.copy(a_sb[:], ps[:])
                            sw = psSW.tile([128, 512], F32, tag="psSW", name="psSW")
                            nc.tensor.matmul(sw[:], pm_t[:], a_sb[:],
                                             start=True, stop=True)
                            t1 = sbA.tile([128, 512], BF16, tag="t1", name="t1")
                            nc.vector.tensor_mul(t1[:], a_sb[:], c_t[:, qs])
                            t2 = sbA.tile([128, 512], BF16, tag="t2", name="t2")
                            nc.vector.tensor_mul(t2[:], sw[:], s_t[:, qs])
                            nc.vector.tensor_add(dst[p][:, qs], t1[:], t2[:])
                    # V projection (natural layout) into V' tiles
                    for j in range(4):
                        sb = c * 4 + j
                        ssl = slice(sb * 128, (sb + 1) * 128)
                        ps = psV.tile([128, HPG * HD], F32, tag="psV", name="psV")
                        for it in range(NIT):
                            nc.tensor.matmul(ps[:], xts[it][:, ssl], wv_t[it][:],
                                             start=(it == 0), stop=(it == NIT - 1))
                        for p in range(NPAIR):
                            src = ps[:, p * 128:(p + 1) * 128]
                            nc.vector.tensor_copy(vp[p][sb][:, 0:64], src[:, 0:64])
                            nc.vector.tensor_copy(vp[p][sb][:, 65:129], src[:, 64:128])

            if debug:
                nc.sync.dma_start(out=dbg["dqt"][:], in_=qt[0][:])
                nc.sync.dma_start(out=dbg["dkt"][:], in_=kt[0][:])
                nc.sync.dma_start(out=dbg["dvp"][:], in_=vp[0][0][:])

            # ================= Attention + wo, chunk-interleaved ==========
            with ExitStack() as phc:
                psS = phc.enter_context(
                    tc.tile_pool(name="psS", bufs=2, space="PSUM"))
                psO = phc.enter_context(
                    tc.tile_pool(name="psO", bufs=1, space="PSUM"))
                prb = phc.enter_context(tc.tile_pool(name="prb", bufs=6))
                nrm = phc.enter_context(tc.tile_pool(name="nrm", bufs=3))
                osb = phc.enter_context(tc.tile_pool(name="osb", bufs=4))
                drp = phc.enter_context(
                    tc.tile_pool(name="drp", bufs=4, space="DRAM"))

                def norm_sub(p, q0, ov, h, sub):
                    """attnT[:, sub] = ov_data * recip(denom row 64)."""
                    s0 = q0 + sub * 512
                    dr = drp.tile([1, 512], F32, tag="dr", name="dr")
                    rr = nrm.tile([65, 512], F32, tag="rr", name="rr")
                    # single-partition custom-DVE ops at base 64 misbehave
                    # on HW: copy the denominator row out, broadcast it,
                    # then reciprocal on 64 partitions
                    nc.vector.tensor_copy(rr[64:65, :], ov[h][sub][64:65, :])
                    nc.sync.dma_start(out=dr[:], in_=rr[64:65, :])
                    rbc = nrm.tile([64, 512], F32, tag="rbc", name="rbc")
                    nc.sync.dma_start(
                        out=rbc[:], in_=dr[:].to_broadcast((64, 512)))
                    rrec = nrm.tile([64, 512], F32, tag="rrec", name="rrec")
                    nc.vector.reciprocal_approx_fast(rrec[:], rbc[:])
                    if h == 0:
                        nc.vector.tensor_mul(
                            at[p][0:64, s0:s0 + 512], ov[h][sub][0:64, :],
                            rrec[:])
                    else:
                        atb = nrm.tile([64, 512], BF16, tag="atb", name="atb")
                        nc.vector.tensor_mul(atb[:], ov[h][sub][0:64, :],
                                             rrec[:])
                        nc.sync.dma_start(
                            out=at[p][64:128, s0:s0 + 512], in_=atb[:])

                def wo_block(sb):
                    """out[sb*128:(sb+1)*128, :] = sum_p at[p].T @ wo_p.
                    PSUM comes from the scores tag (one 512-wide bank per
                    output half)."""
                    ssl = slice(sb * 128, (sb + 1) * 128)
                    ps = psS.tile([128, QCH], F32, tag="sc", name="scW")
                    for oc in range(2):
                        osl = slice(oc * 512, (oc + 1) * 512)
                        for p in range(NPAIR):
                            nc.tensor.matmul(
                                ps[:, osl], at[p][:, ssl], wo_t[p][:, osl],
                                start=(p == 0), stop=(p == NPAIR - 1))
                    ob = osb.tile([128, 1024], BF16, tag="osb", name="osb")
                    nc.vector.tensor_copy(ob[:], ps[:])
                    nc.sync.dma_start(out=out_d[ssl, :], in_=ob[:])

                def attention(c, p, tail_wo, wo_list):
                    """Attention for (chunk, pair), depth-1 software
                    pipelined: scores+exp of k-block kb are emitted BEFORE
                    the PV of k-block kb-1, so the in-order PE queue never
                    stalls on the ScalarE exp. PV runs as row-tiled half-K
                    pairs: head0's half targets ov0 while head1's other half
                    targets ov1 concurrently (disjoint row groups + banks).
                    Normalization of each 512-col sub is emitted right after
                    the k-block that completes it."""
                    q0 = c * QCH
                    kb_hi = (c * 8 + 8) if causal else NSB
                    nsub = QCH // 512
                    ov = [[psO.tile([65, 512], F32, tag=f"ov{h}_{s}",
                                    name=f"ov{h}_{s}") for s in range(nsub)]
                          for h in range(2)]
                    last_for = []
                    for sub in range(nsub):
                        if causal:
                            last_for.append(
                                min(kb_hi, (q0 + (sub + 1) * 512) // 128) - 1)
                        else:
                            last_for.append(kb_hi - 1)

                    def emit_scores(kb):
                        k0 = kb * 128
                        trim = max(q0, k0) if causal else q0
                        on_diag = causal and kb >= c * (QCH // 128)
                        pts = [None, None]
                        for h in range(2):
                            hsl = slice(h * 64, (h + 1) * 64)
                            sc = psS.tile([128, QCH], F32, tag="sc", name="sc")
                            # each 512-col sub-MM opens its own PSUM-bank
                            # accumulation group; the diag-mask matmul closes
                            # the group of the bank it lands in
                            diag_sub = (k0 - q0) // 512 if on_diag else -1
                            for sub in range(nsub):
                                a = max(q0 + sub * 512, trim)
                                b_ = q0 + sub * 512 + 512
                                if a >= b_:
                                    continue
                                nc.tensor.matmul(
                                    sc[:, a - q0:b_ - q0],
                                    kt[p][hsl, k0:k0 + 128],
                                    qt[p][hsl, a:b_],
                                    start=True, stop=(sub != diag_sub))
                            if on_diag:
                                # additive causal mask on diag subblock
                                nc.tensor.matmul(
                                    sc[:, k0 - q0:k0 - q0 + 128],
                                    ident_t[:], mdiag_t[:],
                                    start=False, stop=True)
                            # exp (with fused 1/sqrt(hd) scale) -> bf16
                            pt = prb.tile([128, QCH], BF16, tag="prb", name="prb")
                            nc.scalar.activation(
                                pt[:, trim - q0:], sc[:, trim - q0:],
                                AF.Exp, scale=SM_SCALE)
                            pts[h] = pt
                            if debug and p == 0 and c == 0 and kb == 0 \
                                    and h == 0:
                                nc.sync.dma_start(out=dbg["dpt"][:], in_=pt[:])
                        return kb, trim, pts

                    def emit_pv(st):
                        kb, trim, pts = st
                        for sub in range(nsub):
                            a = max(q0 + sub * 512, trim)
                            b_ = q0 + sub * 512 + 512
                            if a >= b_:
                                continue
                            s0 = q0 + sub * 512
                            first = kb == 0
                            last = kb == last_for[sub]
                            for h in range(2):
                                # PV + denominator (M=65: V_h | ones)
                                nc.tensor.matmul(
                                    ov[h][sub][:, a - s0:b_ - s0],
                                    vp[p][kb][:, h * 65:h * 65 + 65],
                                    pts[h][:, a - q0:b_ - q0],
                                    start=first, stop=last)
                        for sub in range(nsub):
                            if kb == last_for[sub]:
                                if debug and p == 0 and c == 0 and sub == 0:
                                    ovb = nrm.tile([65, 512], F32, tag="ovb",
                                                   name="ovb")
                                    nc.vector.tensor_copy(ovb[:], ov[0][sub][:])
                                    nc.sync.dma_start(
                                        out=dbg["dov"][:, 0:512], in_=ovb[:])
                                norm_sub(p, q0, ov, 0, sub)
                                norm_sub(p, q0, ov, 1, sub)

                    # depth-2 pipeline: PV trails scores by two k-blocks so
                    # the PE queue has cover for the previous chunk's
                    # normalization chain; wo blocks of the previous chunk
                    # are spread between early k-blocks as extra PE filler.
                    depth = 1 if tail_wo else 2
                    pend = []
                    for kb in range(kb_hi):
                        st = emit_scores(kb)
                        if wo_list and kb % 2 == 0:
                            wo_block(wo_list.pop(0))
                        if len(pend) == depth:
                            emit_pv(pend.pop(0))
                        pend.append(st)
                    while pend:
                        emit_pv(pend.pop(0))
                    while wo_list:
                        wo_block(wo_list.pop(0))
                    if tail_wo:
                        # last chunk: emit wo per half so the first half's
                        # blocks don't wait on the second half's norm chain
                        for sub in range(nsub):
                            for j in range(4):
                                wo_block((q0 + sub * 512) // 128 + j)

                for c in range(NCHUNK):
                    last = c == NCHUNK - 1
                    prev = (list(range((c - 1) * (QCH // 128),
                                       c * (QCH // 128)))
                            if c > 0 else [])
                    attention(c, 0, tail_wo=False, wo_list=prev)
                    attention(c, 1, tail_wo=last, wo_list=[])
                    if last:
                        if debug:
                            nc.sync.dma_start(out=dbg["dat0"][:], in_=at[0][:])
                            nc.sync.dma_start(out=dbg["dat1"][:], in_=at[1][:])

    nc.compile()
    return nc


WSCALE = 64.0  # fp8 weight scale: q,k,v carry x64; folded into exp scale / wo


def _pair(a):
    """[D, M] -> [D//256][128, 2, M] DoubleRow it-tile pair layout."""
    d, m = a.shape
    return np.ascontiguousarray(
        a.reshape(d // 256, 2, 128, m).transpose(0, 2, 1, 3))


def _host_prep(x, freqs_cos, freqs_sin, wq, wk, wv, wo):
    """Build the 8 per-core input maps (all numpy, bf16 via float32 rounds)."""
    import ml_dtypes

    bf = ml_dtypes.bfloat16
    f8 = ml_dtypes.float8_e4m3
    x = np.ascontiguousarray(x, dtype=np.float32)
    cosT = np.ascontiguousarray(freqs_cos.T, dtype=np.float32)  # [32, S]
    sinT = np.ascontiguousarray(freqs_sin.T, dtype=np.float32)

    c128 = np.tile(cosT, (4, 1)).astype(bf)                     # [128, S]
    s128 = np.tile(np.concatenate([-sinT, sinT], 0), (2, 1)).astype(bf)
    # swap permutation: psum_sw = pmat.T @ A -> sw[m] = A[sigma(m)],
    # sigma swaps the 32-halves within each 64 block.
    pmat = np.zeros((128, 128), dtype=np.float32)
    for m in range(128):
        blk, off = divmod(m, 32)
        pmat[(blk ^ 1) * 32 + off, m] = 1.0
    pmat = pmat.astype(bf)
    ident = np.eye(128, dtype=np.float32).astype(bf)
    # causal diag mask, transposed: mdiagT[k, q] = 0 if k <= q else -1e9
    kk, qq = np.meshgrid(np.arange(128), np.arange(128), indexing="ij")
    mdiagT = np.where(kk <= qq, 0.0, NEG_INF).astype(np.float32).astype(bf)

    # rotate-half row permutation within each head
    rh = np.concatenate([np.arange(0, HD, 2), np.arange(1, HD, 2)])

    xT = [np.ascontiguousarray(x[b].T).astype(bf) for b in range(B)]

    in_maps = []
    for core in range(NCORES):
        b, g = divmod(core, GROUPS)
        heads = [g * HPG + j for j in range(HPG)]
        qrows, vrows = [], []
        for h in heads:
            base = h * HD
            qrows.extend((base + rh).tolist())
            vrows.extend(range(base, base + HD))
        qrows = np.array(qrows)
        vrows = np.array(vrows)
        wqT = np.ascontiguousarray(wq[qrows, :].T).astype(bf)   # [D, 256]
        wkT = np.ascontiguousarray(wk[qrows, :].T).astype(bf)
        wvT = np.ascontiguousarray(wv[vrows, :].T).astype(bf)
        woT = np.ascontiguousarray(wo[:, vrows].T).astype(bf)
        m = {
            "xT": xT[b], "wqT": wqT, "wkT": wkT, "wvT": wvT,
            "woT": woT, "c128": c128, "s128": s128, "pmat": pmat,
            "ident": ident, "mdiagT": mdiagT,
        }
        in_maps.append(m)
    return in_maps


def _mask_kind(mask):
    m = np.asarray(mask).reshape(S, S)
    if not np.any(m):
        return "zeros"
    qq, kk = np.meshgrid(np.arange(S), np.arange(S), indexing="ij")
    causal = np.where(kk <= qq, 0.0, NEG_INF).astype(np.float32)  # [q, k]
    if np.array_equal(m, causal):
        return "causal"
    return "general"


def _reference_host(x, freqs_cos, freqs_sin, mask, wq, wk, wv, wo):
    """Correctness fallback for arbitrary masks (host numpy, float64)."""
    b, s, d = x.shape
    hd = d // H
    xq = (x @ wq.T).reshape(b, s, H, hd)
    xk = (x @ wk.T).reshape(b, s, H, hd)
    xv = (x @ wv.T).reshape(b, s, H, hd)

    def rope(t):
        tr = t.reshape(b, s, H, hd // 2, 2)
        t0, t1 = tr[..., 0], tr[..., 1]
        cos = freqs_cos[None, :, None, :]
        sin = freqs_sin[None, :, None, :]
        return np.stack([t0 * cos - t1 * sin, t0 * sin + t1 * cos],
                        -1).reshape(b, s, H, hd)

    xq, xk = rope(xq), rope(xk)
    sc = np.einsum("bqhd,bkhd->bhqk", xq, xk) / np.sqrt(hd) + mask
    sc = sc - sc.max(-1, keepdims=True)
    e = np.exp(sc)
    pr = e / e.sum(-1, keepdims=True)
    o = np.einsum("bhqk,bkhd->bqhd", pr, xv).reshape(b, s, d)
    return (o @ wo.T).astype(np.float32)


def kernel(x, freqs_cos, freqs_sin, mask, wq, wk, wv, wo):
    kind = _mask_kind(mask)
    if kind == "general":
        return _reference_host(np.asarray(x, np.float64),
                               np.asarray(freqs_cos, np.float64),
                               np.asarray(freqs_sin, np.float64),
                               np.asarray(mask, np.float64),
                               np.asarray(wq, np.float64),
                               np.asarray(wk, np.float64),
                               np.asarray(wv, np.float64),
                               np.asarray(wo, np.float64))

    if kind not in _PROG_CACHE:
        _PROG_CACHE[kind] = _build_program(kind)
    nc = _PROG_CACHE[kind]

    in_maps = _host_prep(np.asarray(x, np.float32),
                         np.asarray(freqs_cos, np.float32),
                         np.asarray(freqs_sin, np.float32),
                         np.asarray(wq, np.float32),
                         np.asarray(wk, np.float32),
                         np.asarray(wv, np.float32),
                         np.asarray(wo, np.float32))
    res = run_bass_kernel_spmd(nc, in_maps, list(range(NCORES)))
    out = np.zeros((B, S, D), dtype=np.float32)
    for core in range(NCORES):
        out[core // GROUPS] += np.asarray(res.results[core]["out"],
                                          dtype=np.float32)
    return out


# revision 48
# speedup vs baseline: 1.0683x; 1.0063x over previous
"""Trainium2 Bass kernel for nn_Attention_12515534700827.

Multi-head causal attention with RoPE: B=2, S=2048, D=1024, H=16, HD=64.
Sharding: 8 cores = 2 (batch) x 4 (head groups of 4 heads). Each core
computes its 4 heads' attention + its slice of the wo projection; the host
sums the 4 partial outputs per batch (the "all-reduce after wo").

All layout transforms (transposes, head permutations for rotate-half RoPE)
are done host-side in numpy; the device kernel sees matmul-ready layouts.
All matmul operands are bf16 (PE runs bf16 at 1 cycle/row for any moving
size, vs fp32r's 4 cycles/row below N=256); accumulation stays fp32 in
PSUM, softmax exp reads fp32 scores.

Per-core dataflow (pair = 2 heads; 2 pairs per core):
  - x streamed in 512-column chunks so projections start ~3us in and the
    PE stays warm (HAM K=8/8).
  - Q^T,K^T computed directly in [head_dim, seq] layout; RoPE =
    A*C + swap(A)*S with swap done by a PE permutation matmul.
  - scores computed transposed [k, q] (2 heads concurrently via PE row
    tiles), causal k-blocks skipped, diagonal mask added with a bf16
    I.T @ maskT matmul, exp on ScalarE with fused 1/sqrt(hd) scale into
    bf16 probs.
  - attention is software-pipelined (depth 2; depth 1 for the final pair
    so its normalizations emit sooner): scores+exp of k-block kb are
    emitted before the PV of kb-depth, so the in-order PE queue never
    stalls on the ScalarE exp (keeps the PE dense -> HAM at full clock),
    and the previous chunk's wo blocks are spread between early k-blocks
    as PE filler over the normalization-chain wait.
  - PV: probsT [k,q] moving, V' = [V_A|ones|V_B|ones] stationary (M=65
    per head, fused softmax denominator row), accumulated per 512-col sub
    so normalization starts as soon as a sub's causal k-blocks finish.
  - normalization: reciprocal of the denominator row, DMA partition-
    broadcast, multiply into bf16 attnT tiles.
  - wo: emitted one chunk behind attention (after the next chunk's first
    pair) so its dependency on the normalization DMA chain never blocks
    the PE queue; output DMA'd to HBM as bf16, host sums partials in fp32.

Tried and rejected: fp8 DoubleRow projections (V-path quantization alone
gives 4.7e-2 max-rel err -- early queries have a peaked softmax so fp8
noise doesn't average out); diag-mask on VectorE (cross-engine hop stalls
the score pipeline, +43us); PV half-K row-tiled pairs (two in-flight MMs
accumulating the same PSUM bank corrupt results on HW).
"""

import sys

if "/opt/trn_rl_repo" not in sys.path:
    sys.path.insert(0, "/opt/trn_rl_repo")

import numpy as np

import concourse.mybir as mybir
import concourse.tile as tile
from concourse import bacc
from concourse.bass_utils import run_bass_kernel_spmd

F32 = mybir.dt.float32
F8 = mybir.dt.float8e4
BF16 = mybir.dt.bfloat16
AF = mybir.ActivationFunctionType

B, S, D, H, HD = 2, 2048, 1024, 16, 64
NCORES = 8
GROUPS = 4            # head groups (cores per batch)
HPG = H // GROUPS     # heads per core = 4
NPAIR = HPG // 2      # head pairs per core = 2
NEG_INF = -1e9
SM_SCALE = 1.0 / float(np.sqrt(HD))  # 0.125

NIT = D // 128        # 8 contraction tiles
NSB = S // 128        # 16 seq blocks
NC512 = S // 512      # 4
QCH = 1024            # attention q-chunk
NCHUNK = S // QCH     # 2

_PROG_CACHE = {}


def _build_program(mask_kind: str, debug: bool = False):
    """mask_kind: 'causal' (skip + diag mask) or 'zeros' (full, no mask)."""
    causal = mask_kind == "causal"
    nc = bacc.Bacc("TRN2", target_bir_lowering=False, debug=False,
                   num_devices=NCORES)
    dbg = {}

    xT_d = nc.dram_tensor("xT", [D, S], BF16, kind="ExternalInput").ap()
    wqT_d = nc.dram_tensor("wqT", [D, HPG * HD], BF16, kind="ExternalInput").ap()
    wkT_d = nc.dram_tensor("wkT", [D, HPG * HD], BF16, kind="ExternalInput").ap()
    wvT_d = nc.dram_tensor("wvT", [D, HPG * HD], BF16, kind="ExternalInput").ap()
    woT_d = nc.dram_tensor("woT", [HPG * HD, D], BF16, kind="ExternalInput").ap()
    c_d = nc.dram_tensor("c128", [128, S], BF16, kind="ExternalInput").ap()
    s_d = nc.dram_tensor("s128", [128, S], BF16, kind="ExternalInput").ap()
    pmat_d = nc.dram_tensor("pmat", [128, 128], BF16, kind="ExternalInput").ap()
    ident_d = nc.dram_tensor("ident", [128, 128], BF16, kind="ExternalInput").ap()
    mdiag_d = nc.dram_tensor("mdiagT", [128, 128], BF16, kind="ExternalInput").ap()
    out_d = nc.dram_tensor("out", [S, D], BF16, kind="ExternalOutput").ap()
    if debug:
        for nm, shp, dt in [("dqt", [128, S], BF16), ("dkt", [128, S], BF16),
                            ("dvp", [128, 130], BF16), ("dpt", [128, QCH], BF16),
                            ("dov", [65, QCH], F32), ("dat0", [128, S], BF16),
                            ("dat1", [128, S], BF16)]:
            dbg[nm] = nc.dram_tensor(nm, shp, dt, kind="ExternalOutput").ap()

    with tile.TileContext(nc) as tc:
        from contextlib import ExitStack

        with ExitStack() as root:
            pers = root.enter_context(tc.tile_pool(name="pers", bufs=1))

            # ---- persistent SBUF tiles ----
            qt = [pers.tile([128, S], BF16, tag=f"qt{p}", name=f"qt{p}") for p in range(NPAIR)]
            kt = [pers.tile([128, S], BF16, tag=f"kt{p}", name=f"kt{p}") for p in range(NPAIR)]
            # V' per (pair, s-block): [128,130] = V_A|ones|V_B|ones
            vp = [[pers.tile([128, 130], BF16, tag=f"vp{p}_{sb}", name=f"vp{p}_{sb}")
                   for sb in range(NSB)] for p in range(NPAIR)]
            at = [pers.tile([128, S], BF16, tag=f"at{p}", name=f"at{p}") for p in range(NPAIR)]
            wo_t = [pers.tile([128, D], BF16, tag=f"wo{p}", name=f"wo{p}")
                    for p in range(NPAIR)]
            ident_t = pers.tile([128, 128], BF16, tag="ident", name="ident")
            mdiag_t = pers.tile([128, 128], BF16, tag="mdiag", name="mdiag")

            # ---- phase B pools (freed before attention) ----
            with ExitStack() as phb:
                ld = phb.enter_context(tc.tile_pool(name="ld", bufs=1))
                xts = [ld.tile([128, S], BF16, tag=f"xt{it}", name=f"xt{it}") for it in range(NIT)]
                wq_t = [ld.tile([128, HPG * HD], BF16, tag=f"wq{it}", name=f"wq{it}") for it in range(NIT)]
                wk_t = [ld.tile([128, HPG * HD], BF16, tag=f"wk{it}", name=f"wk{it}") for it in range(NIT)]
                wv_t = [ld.tile([128, HPG * HD], BF16, tag=f"wv{it}", name=f"wv{it}") for it in range(NIT)]
                c_t = ld.tile([128, S], BF16, tag="c128", name="c128")
                s_t = ld.tile([128, S], BF16, tag="s128", name="s128")
                pm_t = ld.tile([128, 128], BF16, tag="pmat", name="pmat")

                # DMA issue order tuned so the first QK projection chunk can
                # start ~3us in: q/k weights + rope tables, x chunk 0, then
                # the rest.
                for it in range(NIT):
                    sl = slice(it * 128, (it + 1) * 128)
                    nc.sync.dma_start(out=wq_t[it][:], in_=wqT_d[sl, :])
                    nc.sync.dma_start(out=xts[it][:, 0:512], in_=xT_d[sl, 0:512])
                for it in range(NIT):
                    sl = slice(it * 128, (it + 1) * 128)
                    nc.sync.dma_start(out=wk_t[it][:], in_=wkT_d[sl, :])
                nc.sync.dma_start(out=c_t[:], in_=c_d[:])
                nc.sync.dma_start(out=s_t[:], in_=s_d[:])
                nc.sync.dma_start(out=pm_t[:], in_=pmat_d[:])
                for it in range(NIT):
                    sl = slice(it * 128, (it + 1) * 128)
                    nc.sync.dma_start(out=wv_t[it][:], in_=wvT_d[sl, :])
                nc.sync.dma_start(out=ident_t[:], in_=ident_d[:])
                if causal:
                    nc.sync.dma_start(out=mdiag_t[:], in_=mdiag_d[:])
                for c in range(1, NC512):
                    qs = slice(c * 512, (c + 1) * 512)
                    for it in range(NIT):
                        sl = slice(it * 128, (it + 1) * 128)
                        nc.sync.dma_start(out=xts[it][:, qs], in_=xT_d[sl, qs])
                for p in range(NPAIR):
                    nc.sync.dma_start(
                        out=wo_t[p][:], in_=woT_d[p * 128:(p + 1) * 128, :])

                # ones columns of V' via vector memset
                for p in range(NPAIR):
                    for sb in range(NSB):
                        nc.vector.memset(vp[p][sb][:, 64:65], 1.0)
                        nc.vector.memset(vp[p][sb][:, 129:130], 1.0)

                psA = phb.enter_context(
                    tc.tile_pool(name="psA", bufs=3, space="PSUM"))
                psSW = phb.enter_context(
                    tc.tile_pool(name="psSW", bufs=2, space="PSUM"))
                psV = phb.enter_context(
                    tc.tile_pool(name="psV", bufs=3, space="PSUM"))
                sbA = phb.enter_context(tc.tile_pool(name="sbA", bufs=4))

                # x-chunk-major so each arriving chunk unlocks its work
                for c in range(NC512):
                    qs = slice(c * 512, (c + 1) * 512)
                    # Q/K projections (transposed layout) + rope
                    for wt, dst in ((wq_t, qt), (wk_t, kt)):
                        for p in range(NPAIR):
                            pc = slice(p * 128, (p + 1) * 128)
                            ps = psA.tile([128, 512], F32, tag="psA", name="psA")
                            for it in range(NIT):
                                nc.tensor.matmul(
                                    ps[:], wt[it][:, pc], xts[it][:, qs],
                                    start=(it == 0), stop=(it == NIT - 1))
                            # rope: rot = A*C + swap(A)*S
                            a_sb = sbA.tile([128, 512], BF16, tag="sbA", name="sbA")
                            nc.scalar.copy(a_sb[:], ps[:])
                            sw = psSW.tile([128, 512], F32, tag="psSW", name="psSW")
                            nc.tensor.matmul(sw[:], pm_t[:], a_sb[:],
                                             start=True, stop=True)
                            t1 = sbA.tile([128, 512], BF16, tag="t1", name="t1")
                            nc.vector.tensor_mul(t1[:], a_sb[:], c_t[:, qs])
                            t2 = sbA.tile([128, 512], BF16, tag="t2", name="t2")
                            nc.vector.tensor_mul(t2[:], sw[:], s_t[:, qs])
                            nc.vector.tensor_add(dst[p][:, qs], t1[:], t2[:])
                    # V projection (natural layout) into V' tiles
                    for j in range(4):
                        sb = c * 4 + j
                        ssl = slice(sb * 128, (sb + 1) * 128)
                        ps = psV.tile([128, HPG * HD], F32, tag="psV", name="psV")
                        for it in range(NIT):
                            nc.tensor.matmul(ps[:], xts[it][:, ssl], wv_t[it][:],
                                             start=(it == 0), stop=(it == NIT - 1))
                        for p in range(NPAIR):
                            src = ps[:, p * 128:(p + 1) * 128]
                            nc.vector.tensor_copy(vp[p][sb][:, 0:64], src[:, 0:64])
                            nc.vector.tensor_copy(vp[p][sb][:, 65:129], src[:, 64:128])

            if debug:
                nc.sync.dma_start(out=dbg["dqt"][:], in_=qt[0][:])
                nc.sync.dma_start(out=dbg["dkt"][:], in_=kt[0][:])
                nc.sync.dma_start(out=dbg["dvp"][:], in_=vp[0][0][:])

            # ================= Attention + wo, chunk-interleaved ==========
            with ExitStack() as phc:
                psS = phc.enter_context(
                    tc.tile_pool(name="psS", bufs=2, space="PSUM"))
                psO = phc.enter_context(
                    tc.tile_pool(name="psO", bufs=1, space="PSUM"))
                prb = phc.enter_context(tc.tile_pool(name="prb", bufs=8))
                nrm = phc.enter_context(tc.tile_pool(name="nrm", bufs=3))
                osb = phc.enter_context(tc.tile_pool(name="osb", bufs=4))
                drp = phc.enter_context(
                    tc.tile_pool(name="drp", bufs=4, space="DRAM"))

                def norm_sub(p, q0, ov, h, sub):
                    """attnT[:, sub] = ov_data * recip(denom row 64)."""
                    s0 = q0 + sub * 512
                    dr = drp.tile([1, 512], F32, tag="dr", name="dr")
                    rr = nrm.tile([65, 512], F32, tag="rr", name="rr")
                    # single-partition custom-DVE ops at base 64 misbehave
                    # on HW: copy the denominator row out, broadcast it,
                    # then reciprocal on 64 partitions
                    nc.vector.tensor_copy(rr[64:65, :], ov[h][sub][64:65, :])
                    nc.sync.dma_start(out=dr[:], in_=rr[64:65, :])
                    rbc = nrm.tile([64, 512], F32, tag="rbc", name="rbc")
                    nc.sync.dma_start(
                        out=rbc[:], in_=dr[:].to_broadcast((64, 512)))
                    rrec = nrm.tile([64, 512], F32, tag="rrec", name="rrec")
                    nc.vector.reciprocal_approx_fast(rrec[:], rbc[:])
                    if h == 0:
                        nc.vector.tensor_mul(
                            at[p][0:64, s0:s0 + 512], ov[h][sub][0:64, :],
                            rrec[:])
                    else:
                        atb = nrm.tile([64, 512], BF16, tag="atb", name="atb")
                        nc.vector.tensor_mul(atb[:], ov[h][sub][0:64, :],
                                             rrec[:])
                        nc.sync.dma_start(
                            out=at[p][64:128, s0:s0 + 512], in_=atb[:])

                def wo_block(sb):
                    """out[sb*128:(sb+1)*128, :] = sum_p at[p].T @ wo_p.
                    PSUM comes from the scores tag (one 512-wide bank per
                    output half)."""
                    ssl = slice(sb * 128, (sb + 1) * 128)
                    ps = psS.tile([128, QCH], F32, tag="sc", name="scW")
                    for oc in range(2):
                        osl = slice(oc * 512, (oc + 1) * 512)
                        for p in range(NPAIR):
                            nc.tensor.matmul(
                                ps[:, osl], at[p][:, ssl], wo_t[p][:, osl],
                                start=(p == 0), stop=(p == NPAIR - 1))
                    ob = osb.tile([128, 1024], BF16, tag="osb", name="osb")
                    nc.vector.tensor_copy(ob[:], ps[:])
                    nc.sync.dma_start(out=out_d[ssl, :], in_=ob[:])

                def attention(c, p, tail_wo, wo_list):
                    """Attention for (chunk, pair), depth-1 software
                    pipelined: scores+exp of k-block kb are emitted BEFORE
                    the PV of k-block kb-1, so the in-order PE queue never
                    stalls on the ScalarE exp. PV runs as row-tiled half-K
                    pairs: head0's half targets ov0 while head1's other half
                    targets ov1 concurrently (disjoint row groups + banks).
                    Normalization of each 512-col sub is emitted right after
                    the k-block that completes it."""
                    q0 = c * QCH
                    kb_hi = (c * 8 + 8) if causal else NSB
                    nsub = QCH // 512
                    ov = [[psO.tile([65, 512], F32, tag=f"ov{h}_{s}",
                                    name=f"ov{h}_{s}") for s in range(nsub)]
                          for h in range(2)]
                    last_for = []
                    for sub in range(nsub):
                        if causal:
                            last_for.append(
                                min(kb_hi, (q0 + (sub + 1) * 512) // 128) - 1)
                        else:
                            last_for.append(kb_hi - 1)

                    def emit_scores(kb):
                        k0 = kb * 128
                        trim = max(q0, k0) if causal else q0
                        on_diag = causal and kb >= c * (QCH // 128)
                        pts = [None, None]
                        for h in range(2):
                            hsl = slice(h * 64, (h + 1) * 64)
                            sc = psS.tile([128, QCH], F32, tag="sc", name="sc")
                            # each 512-col sub-MM opens its own PSUM-bank
                            # accumulation group; the diag-mask matmul closes
                            # the group of the bank it lands in
                            diag_sub = (k0 - q0) // 512 if on_diag else -1
                            for sub in range(nsub):
                                a = max(q0 + sub * 512, trim)
                                b_ = q0 + sub * 512 + 512
                                if a >= b_:
                                    continue
                                nc.tensor.matmul(
                                    sc[:, a - q0:b_ - q0],
                                    kt[p][hsl, k0:k0 + 128],
                                    qt[p][hsl, a:b_],
                                    start=True, stop=(sub != diag_sub))
                            if on_diag:
                                # additive causal mask on diag subblock
                                nc.tensor.matmul(
                                    sc[:, k0 - q0:k0 - q0 + 128],
                                    ident_t[:], mdiag_t[:],
                                    start=False, stop=True)
                            # exp (with fused 1/sqrt(hd) scale) -> bf16
                            pt = prb.tile([128, QCH], BF16, tag="prb", name="prb")
                            nc.scalar.activation(
                                pt[:, trim - q0:], sc[:, trim - q0:],
                                AF.Exp, scale=SM_SCALE)
                            pts[h] = pt
                            if debug and p == 0 and c == 0 and kb == 0 \
                                    and h == 0:
                                nc.sync.dma_start(out=dbg["dpt"][:], in_=pt[:])
                        return kb, trim, pts

                    def emit_pv(st):
                        kb, trim, pts = st
                        for sub in range(nsub):
                            a = max(q0 + sub * 512, trim)
                            b_ = q0 + sub * 512 + 512
                            if a >= b_:
                                continue
                            s0 = q0 + sub * 512
                            first = kb == 0
                            last = kb == last_for[sub]
                            for h in range(2):
                                # PV + denominator (M=65: V_h | ones)
                                nc.tensor.matmul(
                                    ov[h][sub][:, a - s0:b_ - s0],
                                    vp[p][kb][:, h * 65:h * 65 + 65],
                                    pts[h][:, a - q0:b_ - q0],
                                    start=first, stop=last)
                        for sub in range(nsub):
                            if kb == last_for[sub]:
                                if debug and p == 0 and c == 0 and sub == 0:
                                    ovb = nrm.tile([65, 512], F32, tag="ovb",
                                                   name="ovb")
                                    nc.vector.tensor_copy(ovb[:], ov[0][sub][:])
                                    nc.sync.dma_start(
                                        out=dbg["dov"][:, 0:512], in_=ovb[:])
                                norm_sub(p, q0, ov, 0, sub)
                                norm_sub(p, q0, ov, 1, sub)

                    # depth-2 pipeline: PV trails scores by two k-blocks so
                    # the PE queue has cover for the previous chunk's
                    # normalization chain; wo blocks of the previous chunk
                    # are spread between early k-blocks as extra PE filler.
                    depth = 1 if tail_wo else 2
                    pend = []
                    for kb in range(kb_hi):
                        st = emit_scores(kb)
                        if wo_list and kb % 2 == 0:
                            wo_block(wo_list.pop(0))
                        if len(pend) == depth:
                            emit_pv(pend.pop(0))
                        pend.append(st)
                    while pend:
                        emit_pv(pend.pop(0))
                    while wo_list:
                        wo_block(wo_list.pop(0))
                    if tail_wo:
                        # last chunk: emit wo per half so the first half's
                        # blocks don't wait on the second half's norm chain
                        for sub in range(nsub):
                            for j in range(4):
                                wo_block((q0 + sub * 512) // 128 + j)

                for c in range(NCHUNK):
                    last = c == NCHUNK - 1
                    prev = (list(range((c - 1) * (QCH // 128),
                                       c * (QCH // 128)))
                            if c > 0 else [])
                    attention(c, 0, tail_wo=False, wo_list=prev)
                    attention(c, 1, tail_wo=last, wo_list=[])
                    if last:
                        if debug:
                            nc.sync.dma_start(out=dbg["dat0"][:], in_=at[0][:])
                            nc.sync.dma_start(out=dbg["dat1"][:], in_=at[1][:])

    nc.compile()
    return nc


WSCALE = 64.0  # fp8 weight scale: q,k,v carry x64; folded into exp scale / wo


def _pair(a):
    """[D, M] -> [D//256][128, 2, M] DoubleRow it-tile pair layout."""
    d, m = a.shape
    return np.ascontiguousarray(
        a.reshape(d // 256, 2, 128, m).transpose(0, 2, 1, 3))


def _host_prep(x, freqs_cos, freqs_sin, wq, wk, wv, wo):
    """Build the 8 per-core input maps (all numpy, bf16 via float32 rounds)."""
    import ml_dtypes

    bf = ml_dtypes.bfloat16
    f8 = ml_dtypes.float8_e4m3
    x = np.ascontiguousarray(x, dtype=np.float32)
    cosT = np.ascontiguousarray(freqs_cos.T, dtype=np.float32)  # [32, S]
    sinT = np.ascontiguousarray(freqs_sin.T, dtype=np.float32)

    c128 = np.tile(cosT, (4, 1)).astype(bf)                     # [128, S]
    s128 = np.tile(np.concatenate([-sinT, sinT], 0), (2, 1)).astype(bf)
    # swap permutation: psum_sw = pmat.T @ A -> sw[m] = A[sigma(m)],
    # sigma swaps the 32-halves within each 64 block.
    pmat = np.zeros((128, 128), dtype=np.float32)
    for m in range(128):
        blk, off = divmod(m, 32)
        pmat[(blk ^ 1) * 32 + off, m] = 1.0
    pmat = pmat.astype(bf)
    ident = np.eye(128, dtype=np.float32).astype(bf)
    # causal diag mask, transposed: mdiagT[k, q] = 0 if k <= q else -1e9
    kk, qq = np.meshgrid(np.arange(128), np.arange(128), indexing="ij")
    mdiagT = np.where(kk <= qq, 0.0, NEG_INF).astype(np.float32).astype(bf)

    # rotate-half row permutation within each head
    rh = np.concatenate([np.arange(0, HD, 2), np.arange(1, HD, 2)])

    xT = [np.ascontiguousarray(x[b].T).astype(bf) for b in range(B)]

    in_maps = []
    for core in range(NCORES):
        b, g = divmod(core, GROUPS)
        heads = [g * HPG + j for j in range(HPG)]
        qrows, vrows = [], []
        for h in heads:
            base = h * HD
            qrows.extend((base + rh).tolist())
            vrows.extend(range(base, base + HD))
        qrows = np.array(qrows)
        vrows = np.array(vrows)
        wqT = np.ascontiguousarray(wq[qrows, :].T).astype(bf)   # [D, 256]
        wkT = np.ascontiguousarray(wk[qrows, :].T).astype(bf)
        wvT = np.ascontiguousarray(wv[vrows, :].T).astype(bf)
        woT = np.ascontiguousarray(wo[:, vrows].T).astype(bf)
        m = {
            "xT": xT[b], "wqT": wqT, "wkT": wkT, "wvT": wvT,
            "woT": woT, "c128": c128, "s128": s128, "pmat": pmat,
            "ident": ident, "mdiagT": mdiagT,
        }
        in_maps.append(m)
    return in_maps


def _mask_kind(mask):
    m = np.asarray(mask).reshape(S, S)
    if not np.any(m):
        return "zeros"
    qq, kk = np.meshgrid(np.arange(S), np.arange(S), indexing="ij")
    causal = np.where(kk <= qq, 0.0, NEG_INF).astype(np.float32)  # [q, k]
    if np.array_equal(m, causal):
        return "causal"
    return "general"


def _reference_host(x, freqs_cos, freqs_sin, mask, wq, wk, wv, wo):
    """Correctness fallback for arbitrary masks (host numpy, float64)."""
    b, s, d = x.shape
    hd = d // H
    xq = (x @ wq.T).reshape(b, s, H, hd)
    xk = (x @ wk.T).reshape(b, s, H, hd)
    xv = (x @ wv.T).reshape(b, s, H, hd)

    def rope(t):
        tr = t.reshape(b, s, H, hd // 2, 2)
        t0, t1 = tr[..., 0], tr[..., 1]
        cos = freqs_cos[None, :, None, :]
        sin = freqs_sin[None, :, None, :]
        return np.stack([t0 * cos - t1 * sin, t0 * sin + t1 * cos],
                        -1).reshape(b, s, H, hd)

    xq, xk = rope(xq), rope(xk)
    sc = np.einsum("bqhd,bkhd->bhqk", xq, xk) / np.sqrt(hd) + mask
    sc = sc - sc.max(-1, keepdims=True)
    e = np.exp(sc)
    pr = e / e.sum(-1, keepdims=True)
    o = np.einsum("bhqk,bkhd->bqhd", pr, xv).reshape(b, s, d)
    return (o @ wo.T).astype(np.float32)


def kernel(x, freqs_cos, freqs_sin, mask, wq, wk, wv, wo):
    kind = _mask_kind(mask)
    if kind == "general":
        return _reference_host(np.asarray(x, np.float64),
                               np.asarray(freqs_cos, np.float64),
                               np.asarray(freqs_sin, np.float64),
                               np.asarray(mask, np.float64),
                               np.asarray(wq, np.float64),
                               np.asarray(wk, np.float64),
                               np.asarray(wv, np.float64),
                               np.asarray(wo, np.float64))

    if kind not in _PROG_CACHE:
        _PROG_CACHE[kind] = _build_program(kind)
    nc = _PROG_CACHE[kind]

    in_maps = _host_prep(np.asarray(x, np.float32),
                         np.asarray(freqs_cos, np.float32),
                         np.asarray(freqs_sin, np.float32),
                         np.asarray(wq, np.float32),
                         np.asarray(wk, np.float32),
                         np.asarray(wv, np.float32),
                         np.asarray(wo, np.float32))
    res = run_bass_kernel_spmd(nc, in_maps, list(range(NCORES)))
    out = np.zeros((B, S, D), dtype=np.float32)
    for core in range(NCORES):
        out[core // GROUPS] += np.asarray(res.results[core]["out"],
                                          dtype=np.float32)
    return out
